# revision 3
# baseline (speedup 1.0000x reference)
"""Trainium2 Bass kernel for CausalSelfAttentionModern (GQA + RoPE + causal SDPA).

Sharding: tensor-parallel over heads across 8 NeuronCores.
Device d owns q-heads {2d, 2d+1} and kv-head d//2.
Each device computes its heads' attention plus its slice of the output
projection (row-parallel); the host sums the 8 partial outputs.

All matmuls run as float32r (full-rate fp32 mode on the PE array).
"""

import numpy as np
import concourse.bacc as bacc
import concourse.tile as tile
import concourse.mybir as mybir
from concourse.bass_utils import run_bass_kernel_spmd

F32 = mybir.dt.float32
F32R = mybir.dt.float32r
EXP = mybir.ActivationFunctionType.Exp

# hardcoded problem shapes
T = 2048          # sequence length
C = 2048          # embedding dim
DH = 128          # head dim
NH = 16           # query heads
NKV = 4           # kv heads
N_CORES = 8
HPD = NH // N_CORES  # q-heads per device = 2
ROPE_BASE = 10000.0
SCALE = 1.0 / np.sqrt(DH)

NQ = 4            # t-quarters for projection phase
TQ = T // NQ      # 512
NW = 4            # attention tq windows
TW = T // NW      # 512
NCT = C // 128    # 16 contraction tiles
NTC = T // 128    # 16 token chunks


def _emit(nc):
    xT = nc.dram_tensor("xT", [C, T], F32R, kind="ExternalInput").ap()
    wq = nc.dram_tensor("wq", [128, NCT * HPD * DH], F32R, kind="ExternalInput").ap()
    wk = nc.dram_tensor("wk", [128, NCT * DH], F32R, kind="ExternalInput").ap()
    wv = nc.dram_tensor("wv", [128, NCT * DH], F32R, kind="ExternalInput").ap()
    wp = nc.dram_tensor("wp", [128, HPD * C], F32R, kind="ExternalInput").ap()
    cosT = nc.dram_tensor("cosT", [128, T], F32, kind="ExternalInput").ap()
    sinR = nc.dram_tensor("sinR", [128, T], F32, kind="ExternalInput").ap()
    ones = nc.dram_tensor("ones", [128, 128], F32R, kind="ExternalInput").ap()
    ident = nc.dram_tensor("ident", [128, 128], F32, kind="ExternalInput").ap()
    out = nc.dram_tensor("out", [T, C], F32, kind="ExternalOutput").ap()

    with tile.TileContext(nc) as tc:
        with (
            tc.tile_pool(name="cst", bufs=1) as cst,
            tc.tile_pool(name="ps", bufs=8, space="PSUM") as ps,
        ):
            # persistent SBUF tensors
            cos_sb = cst.tile([128, T], F32, tag="cos")
            nc.sync.dma_start(cos_sb[:], cosT[:])
            sin_sb = cst.tile([128, T], F32, tag="sin")
            nc.sync.dma_start(sin_sb[:], sinR[:])
            ones_sb = cst.tile([128, 128], F32R, tag="ones")
            nc.sync.dma_start(ones_sb[:], ones[:])
            id_sb = cst.tile([128, 128], F32, tag="ident")
            nc.sync.dma_start(id_sb[:], ident[:])
            wp_sb = cst.tile([128, HPD * C], F32R, tag="wp")
            nc.sync.dma_start(wp_sb[:], wp[:])

            qt_sb = [cst.tile([128, T], F32R, tag=f"qt{m}", name=f"qt{m}") for m in range(HPD)]
            kt_sb = cst.tile([128, T], F32R, tag="kt")
            vt_sb = cst.tile([128, T], F32, tag="vt")
            v_sb = cst.tile([128, NTC * DH], F32R, tag="v")
            yt_sb = [cst.tile([128, T], F32R, tag=f"yt{m}", name=f"yt{m}") for m in range(HPD)]

            # ---------------- projections + rope, per t-quarter ----------------
            with (
                tc.tile_pool(name="wqkv", bufs=1) as wqkv,
                tc.tile_pool(name="xts", bufs=3) as xts,
                tc.tile_pool(name="rope", bufs=2) as rope,
            ):
                wq_sb = wqkv.tile([128, NCT * HPD * DH], F32R, tag="wq")
                nc.sync.dma_start(wq_sb[:], wq[:])
                wk_sb = wqkv.tile([128, NCT * DH], F32R, tag="wk")
                nc.sync.dma_start(wk_sb[:], wk[:])
                wv_sb = wqkv.tile([128, NCT * DH], F32R, tag="wv")
                nc.sync.dma_start(wv_sb[:], wv[:])

                for q in range(NQ):
                    tsl = slice(q * TQ, (q + 1) * TQ)
                    pq = [ps.tile([128, TQ], F32, tag="ps", name=f"pq{q}_{m}") for m in range(HPD)]
                    pk = ps.tile([128, TQ], F32, tag="ps")
                    pv = ps.tile([128, TQ], F32, tag="ps")
                    for ct in range(NCT):
                        xt = xts.tile([128, TQ], F32R, tag="xt")
                        nc.sync.dma_start(xt[:], xT[ct * 128:(ct + 1) * 128, tsl])
                        st = ct == 0
                        sp = ct == NCT - 1
                        for m in range(HPD):
                            nc.tensor.matmul(
                                pq[m][:],
                                wq_sb[:, (ct * HPD + m) * DH:(ct * HPD + m + 1) * DH],
                                xt[:], start=st, stop=sp)
                        nc.tensor.matmul(
                            pk[:], wk_sb[:, ct * DH:(ct + 1) * DH], xt[:],
                            start=st, stop=sp)
                        nc.tensor.matmul(
                            pv[:], wv_sb[:, ct * DH:(ct + 1) * DH], xt[:],
                            start=st, stop=sp)

                    # rope: out = psum*cos + shift(psum)*sinR  (shift = rotate-half)
                    for psrc, dst in [(pq[0], qt_sb[0]), (pq[1], qt_sb[1]), (pk, kt_sb)]:
                        cr = rope.tile([128, TQ], F32, tag="crope")
                        nc.vector.tensor_mul(cr[:], psrc[:], cos_sb[:, tsl])
                        ur = rope.tile([128, TQ], F32, tag="urot")
                        nc.vector.tensor_mul(ur[0:64, :], psrc[64:128, :], sin_sb[0:64, tsl])
                        nc.vector.tensor_mul(ur[64:128, :], psrc[0:64, :], sin_sb[64:128, tsl])
                        nc.vector.tensor_add(dst[:, tsl], cr[:], ur[:])
                    # v: plain copy to SBUF (fp32, feeds PE transpose)
                    nc.scalar.copy(vt_sb[:, tsl], pv[:])

                    # transpose V^T -> V for this quarter (4 token chunks)
                    pvt = ps.tile([128, TQ], F32, tag="ps")
                    for j in range(4):
                        tc_idx = q * 4 + j
                        nc.tensor.transpose(
                            pvt[:, j * 128:(j + 1) * 128],
                            vt_sb[:, tc_idx * 128:(tc_idx + 1) * 128],
                            id_sb[:])
                    nc.vector.tensor_copy(v_sb[:, q * TQ:(q + 1) * TQ], pvt[:])

            # ---------------- attention + output projection, per window ----------------
            with (
                tc.tile_pool(name="pt", bufs=8) as ptp,
                tc.tile_pool(name="rc", bufs=2) as rcp,
                tc.tile_pool(name="ost", bufs=2) as ostp,
            ):
                for w in range(NW):
                    wsl = slice(w * TW, (w + 1) * TW)
                    nch = 4 * (w + 1)
                    for h in range(HPD):
                        y_ps = ps.tile([128, TW], F32, tag="ps")
                        s_ps = ps.tile([128, TW], F32, tag="ps")
                        for cc in range(nch):
                            sc_ps = ps.tile([128, TW], F32, tag="ps")
                            nc.tensor.matmul(
                                sc_ps[:],
                                kt_sb[:, cc * 128:(cc + 1) * 128],
                                qt_sb[h][:, wsl], start=True, stop=True)
                            pt = ptp.tile([128, TW], F32R, tag="pt")
                            nc.scalar.activation(pt[:], sc_ps[:], EXP, scale=float(SCALE))
                            if cc >= 4 * w:
                                # zero strictly-above-diagonal: keep where tq >= tk
                                nc.gpsimd.affine_select(
                                    out=pt[:], in_=pt[:], pattern=[[1, TW]],
                                    compare_op=mybir.AluOpType.is_ge, fill=0.0,
                                    base=w * TW - cc * 128, channel_multiplier=-1)
                            st = cc == 0
                            sp = cc == nch - 1
                            nc.tensor.matmul(
                                y_ps[:], v_sb[:, cc * DH:(cc + 1) * DH], pt[:],
                                start=st, stop=sp)
                            nc.tensor.matmul(
                                s_ps[:], ones_sb[:], pt[:], start=st, stop=sp)
                        rc = rcp.tile([128, TW], F32, tag="rc")
                        nc.vector.reciprocal(rc[:], s_ps[:])
                        nc.vector.tensor_mul(yt_sb[h][:, wsl], y_ps[:], rc[:])

                    # output projection for this window's 4 token chunks
                    for j in range(4):
                        t0 = w * TW + j * 128
                        po = [ps.tile([128, 512], F32, tag="ps", name=f"po{w}_{j}_{e}") for e in range(4)]
                        for k in range(HPD):
                            for e in range(4):
                                nc.tensor.matmul(
                                    po[e][:],
                                    yt_sb[k][:, t0:t0 + 128],
                                    wp_sb[:, k * C + e * 512:k * C + (e + 1) * 512],
                                    start=(k == 0), stop=(k == HPD - 1))
                        ost = ostp.tile([128, C], F32, tag="ost")
                        for e in range(4):
                            nc.any.tensor_copy(ost[:, e * 512:(e + 1) * 512], po[e][:])
                        nc.sync.dma_start(out[t0:t0 + 128, :], ost[:])
    nc.compile()
    return nc


_CACHE = {}


def _get_module():
    if "nc" not in _CACHE:
        nc = bacc.Bacc("TRN2", target_bir_lowering=False, debug=False)
        _CACHE["nc"] = _emit(nc)
    return _CACHE["nc"]


def _host_constants():
    if "consts" in _CACHE:
        return _CACHE["consts"]
    inv_freq = 1.0 / (ROPE_BASE ** (np.arange(0, DH, 2, dtype=np.float64) / DH))
    ang = np.outer(np.arange(T, dtype=np.float64), inv_freq)      # (T, 64)
    emb = np.concatenate([ang, ang], axis=-1)                     # (T, 128)
    cos = np.cos(emb).astype(np.float32)                          # (T, 128)
    sin = np.sin(emb).astype(np.float32)
    cosT = np.ascontiguousarray(cos.T)                            # (128, T)
    sinT = np.ascontiguousarray(sin.T)
    sign = np.where(np.arange(DH) < DH // 2, -1.0, 1.0).astype(np.float32)
    sinR = np.ascontiguousarray(sinT * sign[:, None])
    ones = np.ones((128, 128), dtype=np.float32)
    ident = np.eye(128, dtype=np.float32)
    _CACHE["consts"] = (cosT, sinR, ones, ident)
    return _CACHE["consts"]


def kernel(x, wq, wk, wv, wproj):
    x = np.asarray(x, dtype=np.float32)
    wq = np.asarray(wq, dtype=np.float32)
    wk = np.asarray(wk, dtype=np.float32)
    wv = np.asarray(wv, dtype=np.float32)
    wproj = np.asarray(wproj, dtype=np.float32)

    nc = _get_module()
    cosT, sinR, ones, ident = _host_constants()
    xT = np.ascontiguousarray(x[0].T)                             # (C, T)

    in_maps = []
    for d in range(N_CORES):
        h0 = HPD * d
        g = d // 2
        # wq columns for heads h0..h0+HPD-1 -> [128, NCT*HPD*DH] (c-tile major)
        wq_d = wq[:, h0 * DH:(h0 + HPD) * DH]                     # (C, HPD*DH)
        wq_l = np.ascontiguousarray(
            wq_d.reshape(NCT, 128, HPD * DH).transpose(1, 0, 2).reshape(128, -1))
        wk_d = wk[:, g * DH:(g + 1) * DH]
        wk_l = np.ascontiguousarray(
            wk_d.reshape(NCT, 128, DH).transpose(1, 0, 2).reshape(128, -1))
        wv_d = wv[:, g * DH:(g + 1) * DH]
        wv_l = np.ascontiguousarray(
            wv_d.reshape(NCT, 128, DH).transpose(1, 0, 2).reshape(128, -1))
        # wproj rows for our heads -> [128, HPD*C] (head-major free dim)
        wp_d = wproj[h0 * DH:(h0 + HPD) * DH, :]                  # (HPD*DH, C)
        wp_l = np.ascontiguousarray(
            wp_d.reshape(HPD, 128, C).transpose(1, 0, 2).reshape(128, -1))
        in_maps.append({
            "xT": xT, "wq": wq_l, "wk": wk_l, "wv": wv_l, "wp": wp_l,
            "cosT": cosT, "sinR": sinR, "ones": ones, "ident": ident,
        })

    res = run_bass_kernel_spmd(nc, in_maps, core_ids=list(range(N_CORES)))
    acc = res.results[0]["out"].astype(np.float32)
    for d in range(1, N_CORES):
        acc = acc + res.results[d]["out"]
    return acc.reshape(1, T, C)


# revision 12
# speedup vs baseline: 1.1871x; 1.1871x over previous
"""Trainium2 Bass kernel for CausalSelfAttentionModern (GQA + RoPE + causal SDPA).

Sharding: tensor-parallel over heads across 8 NeuronCores.
Device d owns q-heads {2d, 2d+1} and kv-head d//2.
Each device computes its heads' attention plus its slice of the output
projection (row-parallel); the host sums the 8 partial outputs.

All matmuls run as float32r (full-rate fp32 mode on the PE array).
"""

import numpy as np
import concourse.bacc as bacc
import concourse.tile as tile
import concourse.mybir as mybir
from concourse.bass_utils import run_bass_kernel_spmd

F32 = mybir.dt.float32
F32R = mybir.dt.float32r
EXP = mybir.ActivationFunctionType.Exp

# hardcoded problem shapes
T = 2048          # sequence length
C = 2048          # embedding dim
DH = 128          # head dim
NH = 16           # query heads
NKV = 4           # kv heads
N_CORES = 8
HPD = NH // N_CORES  # q-heads per device = 2
ROPE_BASE = 10000.0
SCALE = 1.0 / np.sqrt(DH)

NQ = 4            # t-quarters for projection phase
TQ = T // NQ      # 512
NW = 4            # attention tq windows
TW = T // NW      # 512
NCT = C // 128    # 16 contraction tiles
NTC = T // 128    # 16 token chunks


def _emit(nc):
    xT = nc.dram_tensor("xT", [C, T], F32R, kind="ExternalInput").ap()
    wq = nc.dram_tensor("wq", [128, NCT * HPD * DH], F32R, kind="ExternalInput").ap()
    wk = nc.dram_tensor("wk", [128, NCT * DH], F32R, kind="ExternalInput").ap()
    wv = nc.dram_tensor("wv", [128, NCT * DH], F32R, kind="ExternalInput").ap()
    wp = nc.dram_tensor("wp", [128, HPD * C], F32R, kind="ExternalInput").ap()
    cosT = nc.dram_tensor("cosT", [128, T], F32, kind="ExternalInput").ap()
    sinR = nc.dram_tensor("sinR", [128, T], F32, kind="ExternalInput").ap()
    ones = nc.dram_tensor("ones", [128, 128], F32R, kind="ExternalInput").ap()
    ident = nc.dram_tensor("ident", [128, 128], F32, kind="ExternalInput").ap()
    out = nc.dram_tensor("out", [T, C], mybir.dt.bfloat16, kind="ExternalOutput").ap()

    with tile.TileContext(nc) as tc:
        with (
            tc.tile_pool(name="cst", bufs=1) as cst,
            tc.tile_pool(name="ps", bufs=8, space="PSUM") as ps,
        ):
            # persistent SBUF tensors (DMAs emitted at first-use points below)
            cos_sb = cst.tile([128, T], F32, tag="cos")
            sin_sb = cst.tile([128, T], F32, tag="sin")
            ones_sb = cst.tile([128, 128], F32R, tag="ones")
            id_sb = cst.tile([128, 128], F32, tag="ident")
            wp_sb = cst.tile([128, HPD * C], F32R, tag="wp")

            qt_sb = [cst.tile([128, T], F32R, tag=f"qt{m}", name=f"qt{m}")
                     for m in range(HPD)]
            kt_sb = cst.tile([128, T], F32R, tag="kt")
            vtp_pool = None  # vt quarter tiles come from the rope pool
            v_sb = cst.tile([128, NTC * DH], F32R, tag="v")
            yt_sb = [cst.tile([128, T], F32R, tag=f"yt{m}", name=f"yt{m}")
                     for m in range(HPD)]

            # ---------------- projections + rope, per t-quarter ----------------
            with (
                tc.tile_pool(name="wqkv", bufs=1) as wqkv,
                tc.tile_pool(name="xts", bufs=4) as xts,
                tc.tile_pool(name="rope", bufs=1) as rope,
            ):
                wq_sb = wqkv.tile([128, NCT * HPD * DH], F32R, tag="wq")
                wk_sb = wqkv.tile([128, NCT * DH], F32R, tag="wk")
                wv_sb = wqkv.tile([128, NCT * DH], F32R, tag="wv")
                # weights on the ACT ring: first c-tile small+fast, rest bulk
                q1 = HPD * DH
                nc.scalar.dma_start(wq_sb[:, 0:q1], wq[:, 0:q1])
                nc.scalar.dma_start(wk_sb[:, 0:DH], wk[:, 0:DH])
                nc.scalar.dma_start(wv_sb[:, 0:DH], wv[:, 0:DH])
                nc.scalar.dma_start(wq_sb[:, q1:], wq[:, q1:])
                nc.scalar.dma_start(wk_sb[:, DH:], wk[:, DH:])
                nc.scalar.dma_start(wv_sb[:, DH:], wv[:, DH:])

                xt_tiles = {}
                vt_tiles = {}

                def emit_xt_loads(qq):
                    # per half (8 c-tiles x 256 tokens) strided load
                    SW = TQ // 2
                    tsl = slice(qq * SW, (qq + 1) * SW)
                    for half in range(2):
                        xt = xts.tile([128, 8 * SW], F32R, tag="xt",
                                      name=f"xtq{qq}_{half}")
                        c0 = half * 8
                        splits = [(0, 4), (4, 8)] if (qq == 0 and half == 0) else [(0, 8)]
                        for a, b in splits:
                            nc.sync.dma_start(
                                xt[:, a * SW:b * SW].rearrange("p (ct t) -> p ct t", t=SW),
                                xT[(c0 + a) * 128:(c0 + b) * 128, tsl].rearrange(
                                    "(ct p) t -> p ct t", p=128))
                        xt_tiles[(qq, half)] = xt

                def emit_proj_quarter(q):
                    SW = TQ // 2
                    tsl = slice(q * TQ, (q + 1) * TQ)
                    pq = [ps.tile([128, TQ], F32, tag="ps", name=f"pq{q}_{m}")
                          for m in range(HPD)]
                    pk = ps.tile([128, TQ], F32, tag="ps", name=f"pk{q}")
                    pv = ps.tile([128, TQ], F32, tag="ps", name=f"pv{q}")
                    for s in range(2):
                        osl = slice(s * SW, (s + 1) * SW)
                        for ct in range(NCT):
                            xt = xt_tiles[(2 * q + s, ct // 8)]
                            xsl = slice((ct % 8) * SW, (ct % 8 + 1) * SW)
                            st = ct == 0
                            sp = ct == NCT - 1
                            for m in range(HPD):
                                nc.tensor.matmul(
                                    pq[m][:, osl],
                                    wq_sb[:, (ct * HPD + m) * DH:(ct * HPD + m + 1) * DH],
                                    xt[:, xsl], start=st, stop=sp)
                            nc.tensor.matmul(
                                pk[:, osl], wk_sb[:, ct * DH:(ct + 1) * DH],
                                xt[:, xsl], start=st, stop=sp)
                            nc.tensor.matmul(
                                pv[:, osl], wv_sb[:, ct * DH:(ct + 1) * DH],
                                xt[:, xsl], start=st, stop=sp)

                    if q == 0:
                        # constants needed from the rope/attention phases on
                        nc.scalar.dma_start(cos_sb[:], cosT[:])
                        nc.scalar.dma_start(sin_sb[:], sinR[:])
                        nc.scalar.dma_start(id_sb[:], ident[:])
                        nc.scalar.dma_start(ones_sb[:], ones[:])
                    if q == 1:
                        nc.scalar.dma_start(wp_sb[:], wp[:])

                    # rope: out = psum*cos + shift(psum)*sinR  (shift = rotate-half)
                    for psrc, dst in [(pq[0], qt_sb[0]), (pq[1], qt_sb[1]), (pk, kt_sb)]:
                        cr = rope.tile([128, TQ], F32, tag="crope")
                        nc.vector.tensor_mul(cr[:], psrc[:], cos_sb[:, tsl])
                        ur = rope.tile([128, TQ], F32, tag="urot")
                        nc.vector.tensor_mul(ur[0:64, :], psrc[64:128, :], sin_sb[0:64, tsl])
                        nc.vector.tensor_mul(ur[64:128, :], psrc[0:64, :], sin_sb[64:128, tsl])
                        nc.vector.tensor_add(dst[:, tsl], cr[:], ur[:])
                    # v: plain copy to SBUF (fp32, feeds PE transpose)
                    vt_q = rope.tile([128, TQ], F32, tag="vtq", name=f"vtq{q}")
                    nc.scalar.copy(vt_q[:], pv[:])
                    vt_tiles[q] = vt_q

                def emit_v_transpose(q):
                    # transpose V^T -> V for quarter q (4 token chunks)
                    pvt = ps.tile([128, TQ], F32, tag="ps", name=f"pvt{q}")
                    for j in range(4):
                        nc.tensor.transpose(
                            pvt[:, j * 128:(j + 1) * 128],
                            vt_tiles[q][:, j * 128:(j + 1) * 128],
                            id_sb[:])
                    nc.vector.tensor_copy(v_sb[:, q * TQ:(q + 1) * TQ], pvt[:])

                def emit_attn_window(tw0, twl):
                    wsl = slice(tw0, tw0 + twl)
                    nch = (tw0 + twl) // 128
                    w = tw0 // 128  # first diagonal chunk index
                    for h in range(HPD):
                        # phase 1: scores -> exp -> causal-zero, decoupled from pV
                        pts = []
                        for cc in range(nch):
                            sc_ps = ps.tile([128, twl], F32, tag="ps",
                                            name=f"sc{w}_{h}_{cc}")
                            nc.tensor.matmul(
                                sc_ps[:],
                                kt_sb[:, cc * 128:(cc + 1) * 128],
                                qt_sb[h][:, wsl], start=True, stop=True)
                            pt = ptp.tile([128, twl], F32R, tag="pt",
                                          name=f"pt{w}_{h}_{cc}")
                            nc.scalar.activation(pt[:], sc_ps[:], EXP, scale=float(SCALE))
                            if cc >= w:
                                # zero strictly-above-diagonal: keep where tq >= tk
                                nc.gpsimd.affine_select(
                                    out=pt[:], in_=pt[:], pattern=[[1, twl]],
                                    compare_op=mybir.AluOpType.is_ge, fill=0.0,
                                    base=tw0 - cc * 128, channel_multiplier=-1)
                            pts.append(pt)
                        # phase 2: y^T += V^T-chunks @ probs, sums via ones-matmul
                        y_ps = ps.tile([128, twl], F32, tag="ps", name=f"y{w}_{h}")
                        s_ps = ps.tile([128, twl], F32, tag="ps", name=f"s{w}_{h}")
                        for cc in range(nch):
                            st = cc == 0
                            sp = cc == nch - 1
                            nc.tensor.matmul(
                                y_ps[:], v_sb[:, cc * DH:(cc + 1) * DH], pts[cc][:],
                                start=st, stop=sp)
                            nc.tensor.matmul(
                                s_ps[:], ones_sb[:], pts[cc][:], start=st, stop=sp)
                        rc = rcp.tile([128, twl], F32, tag="rc", name=f"rc{w}_{h}")
                        nc.vector.reciprocal(rc[:], s_ps[:])
                        nc.vector.tensor_mul(yt_sb[h][:, wsl], y_ps[:], rc[:])

                def emit_outproj_window(tw0, twl):
                    for j in range(twl // 128):
                        t0 = tw0 + j * 128
                        po = [ps.tile([128, 512], F32, tag="ps",
                                      name=f"po{t0}_{e}") for e in range(4)]
                        for k in range(HPD):
                            for e in range(4):
                                nc.tensor.matmul(
                                    po[e][:],
                                    yt_sb[k][:, t0:t0 + 128],
                                    wp_sb[:, k * C + e * 512:k * C + (e + 1) * 512],
                                    start=(k == 0), stop=(k == HPD - 1))
                        ost = ostp.tile([128, C], mybir.dt.bfloat16, tag="ost", name=f"ost{t0}")
                        for e in range(4):
                            nc.any.tensor_copy(ost[:, e * 512:(e + 1) * 512], po[e][:])
                        nc.sync.dma_start(out[t0:t0 + 128, :], ost[:])

                with (
                    tc.tile_pool(name="pt", bufs=14) as ptp,
                    tc.tile_pool(name="rc", bufs=2) as rcp,
                    tc.tile_pool(name="ost", bufs=2) as ostp,
                ):
                    for qq in range(2 * NQ):
                        emit_xt_loads(qq)
                    for q in range(NQ):
                        emit_proj_quarter(q)
                        if q >= 2:
                            emit_outproj_window((q - 2) * TW, TW)
                        if q >= 1:
                            emit_v_transpose(q - 1)
                            emit_attn_window((q - 1) * TW, TW)
                    emit_v_transpose(NQ - 1)
                    emit_outproj_window((NQ - 2) * TW, TW)
                    emit_attn_window(3 * TW, TW // 2)
                    emit_outproj_window(3 * TW, TW // 2)
                    emit_attn_window(3 * TW + TW // 2, TW // 2)
                    emit_outproj_window(3 * TW + TW // 2, TW // 2)

    nc.compile()
    return nc


_CACHE = {}


def _get_module():
    if "nc" not in _CACHE:
        nc = bacc.Bacc("TRN2", target_bir_lowering=False, debug=False)
        _CACHE["nc"] = _emit(nc)
    return _CACHE["nc"]


def _host_constants():
    if "consts" in _CACHE:
        return _CACHE["consts"]
    inv_freq = 1.0 / (ROPE_BASE ** (np.arange(0, DH, 2, dtype=np.float64) / DH))
    ang = np.outer(np.arange(T, dtype=np.float64), inv_freq)      # (T, 64)
    emb = np.concatenate([ang, ang], axis=-1)                     # (T, 128)
    cos = np.cos(emb).astype(np.float32)                          # (T, 128)
    sin = np.sin(emb).astype(np.float32)
    cosT = np.ascontiguousarray(cos.T)                            # (128, T)
    sinT = np.ascontiguousarray(sin.T)
    sign = np.where(np.arange(DH) < DH // 2, -1.0, 1.0).astype(np.float32)
    sinR = np.ascontiguousarray(sinT * sign[:, None])
    ones = np.ones((128, 128), dtype=np.float32)
    ident = np.eye(128, dtype=np.float32)
    _CACHE["consts"] = (cosT, sinR, ones, ident)
    return _CACHE["consts"]


def kernel(x, wq, wk, wv, wproj):
    x = np.asarray(x, dtype=np.float32)
    wq = np.asarray(wq, dtype=np.float32)
    wk = np.asarray(wk, dtype=np.float32)
    wv = np.asarray(wv, dtype=np.float32)
    wproj = np.asarray(wproj, dtype=np.float32)

    nc = _get_module()
    cosT, sinR, ones, ident = _host_constants()
    xT = np.ascontiguousarray(x[0].T)                             # (C, T)

    in_maps = []
    for d in range(N_CORES):
        h0 = HPD * d
        g = d // 2
        # wq columns for heads h0..h0+HPD-1 -> [128, NCT*HPD*DH] (c-tile major)
        wq_d = wq[:, h0 * DH:(h0 + HPD) * DH]                     # (C, HPD*DH)
        wq_l = np.ascontiguousarray(
            wq_d.reshape(NCT, 128, HPD * DH).transpose(1, 0, 2).reshape(128, -1))
        wk_d = wk[:, g * DH:(g + 1) * DH]
        wk_l = np.ascontiguousarray(
            wk_d.reshape(NCT, 128, DH).transpose(1, 0, 2).reshape(128, -1))
        wv_d = wv[:, g * DH:(g + 1) * DH]
        wv_l = np.ascontiguousarray(
            wv_d.reshape(NCT, 128, DH).transpose(1, 0, 2).reshape(128, -1))
        # wproj rows for our heads -> [128, HPD*C] (head-major free dim)
        wp_d = wproj[h0 * DH:(h0 + HPD) * DH, :]                  # (HPD*DH, C)
        wp_l = np.ascontiguousarray(
            wp_d.reshape(HPD, 128, C).transpose(1, 0, 2).reshape(128, -1))
        in_maps.append({
            "xT": xT, "wq": wq_l, "wk": wk_l, "wv": wv_l, "wp": wp_l,
            "cosT": cosT, "sinR": sinR, "ones": ones, "ident": ident,
        })

    res = run_bass_kernel_spmd(nc, in_maps, core_ids=list(range(N_CORES)))
    acc = res.results[0]["out"].astype(np.float32)
    for d in range(1, N_CORES):
        acc = acc + res.results[d]["out"].astype(np.float32)
    return acc.reshape(1, T, C)
